# revision 18
# baseline (speedup 1.0000x reference)
# CATS-SwiGLU decode kernel for TRN2 (8 NeuronCores, SPMD tensor-parallel).
# v6: mixed-precision weight streaming.  Gate stays fp16 (flag flips near the
# |x1|>thr boundary are the error budget's hard wall); up and down are
# fp8-e3m4 scaled by 128 (the x128 cancels between z=pup*x1/2^14 and the
# scaled down weights).  PE consumes the whole gate+up streams as GEMVs with
# moving weights; DVE handles 8/32 of the down d-groups plus the elementwise
# chain.
#
# Up and gate pieces are interleaved in both DMA-ring order and PE queue
# order so the PE runs dense (no HAM re-throttle) and the 6-deep gate pool
# never stalls the rings; the first pieces are small so the PE starts ~6us
# earlier.  zrep is built by a partition-broadcast SBUF->SBUF DMA on the
# (by then idle) sync ring instead of PE ones-matmuls, so the PE tail is
# purely down-projection GEMVs.  The down stream is pinned behind the last
# gate piece by the dummy-DMA WAW trick.
import sys

for _p in ("/opt/trn_rl_repo",):
    if _p not in sys.path:
        sys.path.insert(0, _p)

import numpy as np
import ml_dtypes

import concourse.bass as bass
import concourse.tile as tile
from concourse import bacc, mybir
from concourse.bass_utils import run_bass_kernel_spmd
from concourse.masks import make_identity

D = 4096
FF = 11008
NCORES = 8
FSH = FF // NCORES            # 1376 f-rows per core
NCD = D // 128                # 32 d-chunks (contraction for gate/up)
NCF = (FSH + 127) // 128      # 11 f-chunks
LASTF = FSH - 128 * (NCF - 1)  # 96 rows in the last f chunk
FPAD = NCF * 128              # 1408 padded f-rows for wdp

DP = 2944                     # d-cols on PE for down
NDVG = (D - DP) // 128        # 9 DVE down groups
FT = ((0, 512), (512, 512), (1024, 352))          # f tiles of gate/up psum rows
DT = ((0, 512), (512, 512), (1024, 512), (1536, 512), (2048, 512), (2560, 384))
ZS = ((0, 512, 0, 4), (512, 512, 4, 4), (1024, 352, 8, 3))  # z_row slices

WSCL = 128.0                  # e3m4 weight scale; 1/WSCL^2 folded into u_row
UPC = (1, 1, 2, 4, 4, 4, 4, 4, 4, 4)   # up piece sizes in chunks
GPC = (2, 2, 4, 4, 4, 4, 4, 4, 4)      # gate piece sizes in chunks
GBUFS = 6                     # gate rotating pool depth (pieces)
WDDPC = (3, 3, 3)             # wdd pieces, in d-groups
WDPPC = (3, 3, 3, 2)          # wdp pieces, in f-chunks

F32 = mybir.dt.float32
F16 = mybir.dt.float16
FP8 = mybir.dt.float8e3
NF16 = np.float16
NF8 = ml_dtypes.float8_e3m4

_CACHE = {}


def _bcast(ap, parts):
    return bass.AP(tensor=ap.tensor, offset=ap.offset, ap=[[0, parts]] + list(ap.ap))


def _row_bcast(sl_ap, parts=128):
    # partition-broadcast of a [1, N] SBUF row: 128 descriptors reading the
    # same row
    return bass.AP(
        tensor=sl_ap.tensor,
        offset=sl_ap.offset,
        ap=[[0, parts]] + list(sl_ap.ap)[1:],
    )


def _pieces():
    """Global interleaved piece order: (kind, chunk0, nchunks).
    Two up pieces first (prime the PE), then strict u/g alternation."""
    seq = []
    ui = gi = 0
    uoff = goff = 0
    for k in range(len(UPC) + len(GPC)):
        want_u = (k < 2) or (k % 2 == 1)
        if (want_u and ui < len(UPC)) or gi >= len(GPC):
            seq.append(("u", uoff, UPC[ui]))
            uoff += UPC[ui]
            ui += 1
        else:
            seq.append(("g", goff, GPC[gi]))
            goff += GPC[gi]
            gi += 1
    assert uoff == NCD and goff == NCD
    return seq


def _build_nc():
    nc = bacc.Bacc("TRN2", target_bir_lowering=False, debug=False)

    xc_d = nc.dram_tensor("xc", [128, NCD], F16, kind="ExternalInput")
    wu_d = nc.dram_tensor("wu", [128, NCD * FSH], FP8, kind="ExternalInput")
    wg_d = nc.dram_tensor("wg", [128, NCD * FSH], F16, kind="ExternalInput")
    wdp_d = nc.dram_tensor("wdp", [128, NCF * DP], FP8, kind="ExternalInput")
    wdd_d = nc.dram_tensor("wdd", [128, NDVG * FSH], FP8, kind="ExternalInput")
    thr_d = nc.dram_tensor("thr", [1], F32, kind="ExternalInput")
    outp_d = nc.dram_tensor("outp", [DP], F32, kind="ExternalOutput")
    outd_d = nc.dram_tensor("outd", [128, NDVG], F32, kind="ExternalOutput")

    with tile.TileContext(nc) as tc:
        with (
            tc.tile_pool(name="const", bufs=1) as cp,
            tc.tile_pool(name="gpool", bufs=GBUFS) as gpool,
            tc.tile_pool(name="acts", bufs=1) as acts,
        ):
            # x column tile rides first on the sync ring; thr on scalar.
            xcol = cp.tile([128, NCD], F16)
            nc.sync.dma_start(out=xcol[:], in_=xc_d.ap())
            thr_sb = cp.tile([128, 1], F32)
            nc.scalar.dma_start(out=thr_sb[:], in_=_bcast(thr_d.ap(), 128))
            ones_f = cp.tile([1, 1], F16)
            nc.vector.memset(ones_f[:], 1.0)
            ones_c = cp.tile([1, 128], F16)
            nc.vector.memset(ones_c[:], 1.0)
            ident = cp.tile([128, 128], F16)
            make_identity(nc, ident[:])

            # ACT warmups: preload the Silu/Abs tables before the hot path.
            warm = acts.tile([128, 1], F32)
            nc.scalar.activation(
                warm[:], thr_sb[:], mybir.ActivationFunctionType.Silu
            )
            nc.scalar.activation(
                warm[:], thr_sb[:], mybir.ActivationFunctionType.Abs
            )
            nc.scalar.copy(warm[:], thr_sb[:])

            # resident weight tiles
            wu_sb = acts.tile([128, NCD * FSH], FP8)
            wdp_sb = acts.tile([128, NCF * DP], FP8)
            wdd_sb = acts.tile([128, NDVG * FSH], FP8)

            # activation scratch
            g_row = acts.tile([1, FSH], F16)
            u_row = acts.tile([1, FSH], F16)
            dve_scr = acts.tile([128, FSH], F16)
            trig8 = acts.tile([1, 8], FP8)
            x1c = acts.tile([128, NCF], F32)
            abc = acts.tile([128, NCF], F32)
            mkc = acts.tile([128, NCF], F32)
            xmc = acts.tile([128, NCF], F32)
            u_col = acts.tile([128, NCF], F32)
            z_col = acts.tile([128, NCF], F16)
            z_row = acts.tile([1, FSH], F16)
            zrep = acts.tile([128, FSH], F16)
            osb = acts.tile([1, DP], F32)
            outd_sb = acts.tile([128, NDVG], F32)

            # ---- up + gate weight stream, interleaved across sync/gpsimd ----
            qs = (nc.sync, nc.gpsimd)
            seq = _pieces()
            gtiles = []
            for k, (kind, c0, ncc) in enumerate(seq):
                sl = slice(c0 * FSH, (c0 + ncc) * FSH)
                q = qs[k % 2]
                if kind == "u":
                    q.dma_start(out=wu_sb[:, sl], in_=wu_d.ap()[:, sl])
                else:
                    t = gpool.tile([128, 4 * FSH], F16, tag="gw", name="gw")
                    q.dma_start(
                        out=t[:, 0 : ncc * FSH], in_=wg_d.ap()[:, sl]
                    )
                    gtiles.append((t, c0, ncc))

            with tc.tile_pool(name="ps1", bufs=1, space="PSUM") as ps1:
                pup = ps1.tile([1, 1536], F32)
                pgate = ps1.tile([1, 1536], F32)
                pzcu = ps1.tile([128, 16], F32)
                pzcg = ps1.tile([128, 16], F32)

                # PE: up+gate GEMVs in the same interleaved order as arrival
                gidx = 0
                for kind, c0, ncc in seq:
                    if kind == "u":
                        for cc in range(ncc):
                            c = c0 + cc
                            for toff, tlen in FT:
                                nc.tensor.matmul(
                                    out=pup[0:1, toff : toff + tlen],
                                    lhsT=xcol[:, c : c + 1],
                                    rhs=wu_sb[
                                        :, c * FSH + toff : c * FSH + toff + tlen
                                    ],
                                    start=(c == 0),
                                    stop=(c == NCD - 1),
                                )
                    else:
                        t, tc0, tncc = gtiles[gidx]
                        gidx += 1
                        for cc in range(tncc):
                            c = tc0 + cc
                            for toff, tlen in FT:
                                nc.tensor.matmul(
                                    out=pgate[0:1, toff : toff + tlen],
                                    lhsT=xcol[:, c : c + 1],
                                    rhs=t[
                                        :, cc * FSH + toff : cc * FSH + toff + tlen
                                    ],
                                    start=(c == 0),
                                    stop=(c == NCD - 1),
                                )

                # ---- down-stream DMAs, pinned behind the last gate piece ----
                trig = gtiles[-1][0]
                nc.scalar.copy(trig8[0:1, 0:8], trig[0:1, 0:8])
                nc.scalar.dma_start(
                    out=wdd_sb[0:1, 0 : 2 * 3 * FSH + 1 : 3 * FSH],
                    in_=trig8[0:1, 0:3],
                )
                nc.scalar.dma_start(
                    out=wdp_sb[0:1, 0 : 3 * 3 * DP + 1 : 3 * DP],
                    in_=trig8[0:1, 3:7],
                )
                do = 0
                for npc in WDDPC:
                    w = npc * FSH
                    nc.scalar.dma_start(
                        out=wdd_sb[:, do : do + w],
                        in_=wdd_d.ap()[:, do : do + w],
                    )
                    do += w
                po = 0
                for npc in WDPPC:
                    w = npc * DP
                    nc.scalar.dma_start(
                        out=wdp_sb[:, po : po + w],
                        in_=wdp_d.ap()[:, po : po + w],
                    )
                    po += w

                # ---- u elementwise (off critical path) ----
                # u_row = pup / WSCL^2 (fold both e3m4 scales); ACT+DVE halves
                nc.scalar.activation(
                    u_row[0:1, 0:688], pup[0:1, 0:688],
                    mybir.ActivationFunctionType.Copy,
                    scale=1.0 / (WSCL * WSCL),
                )
                nc.vector.tensor_scalar(
                    out=u_row[0:1, 688:FSH], in0=pup[0:1, 688:FSH],
                    scalar1=1.0 / (WSCL * WSCL), scalar2=None,
                    op0=mybir.AluOpType.mult,
                )
                for c in range(NCF):
                    pc = 128 if c < NCF - 1 else LASTF
                    nc.tensor.matmul(
                        out=pzcu[0:pc, c : c + 1],
                        lhsT=u_row[0:1, c * 128 : c * 128 + pc],
                        rhs=ones_f[0:1, 0:1],
                        start=True,
                        stop=True,
                    )
                nc.scalar.copy(u_col[:, 0:NCF], pzcu[:, 0:NCF])

                # ---- gate elementwise (z critical path) ----
                nc.scalar.copy(g_row[0:1, 0:688], pgate[0:1, 0:688])
                nc.vector.tensor_copy(g_row[0:1, 688:FSH], pgate[0:1, 688:FSH])
                for c in range(NCF):
                    pc = 128 if c < NCF - 1 else LASTF
                    nc.tensor.matmul(
                        out=pzcg[0:pc, c : c + 1],
                        lhsT=g_row[0:1, c * 128 : c * 128 + pc],
                        rhs=ones_f[0:1, 0:1],
                        start=True,
                        stop=True,
                    )
                nc.scalar.activation(
                    x1c[:, 0:NCF], pzcg[:, 0:NCF],
                    mybir.ActivationFunctionType.Silu,
                )
                nc.scalar.activation(
                    abc[:, 0:NCF], x1c[:, 0:NCF],
                    mybir.ActivationFunctionType.Abs,
                )
                nc.vector.tensor_scalar(
                    out=mkc[:, 0:NCF], in0=abc[:, 0:NCF],
                    scalar1=thr_sb[:], scalar2=None,
                    op0=mybir.AluOpType.is_gt,
                )
                nc.vector.tensor_mul(xmc[:, 0:NCF], x1c[:, 0:NCF], mkc[:, 0:NCF])
                nc.vector.tensor_mul(z_col[:, 0:NCF], u_col[:, 0:NCF], xmc[:, 0:NCF])

            with tc.tile_pool(name="ps2", bufs=1, space="PSUM") as ps2:
                pzrow = ps2.tile([1, 512], F32)
                przep = ps2.tile([128, 512], F32)
                pdp = ps2.tile([1, DP], F32)

                # z_row slices via PE transposes; zrep slices via PE
                # ones-broadcast matmuls (sharing one psum bank pair).
                for soff, slen, c0, ncc in ZS:
                    for i in range(ncc):
                        c = c0 + i
                        pc = 128 if c < NCF - 1 else LASTF
                        nc.tensor.matmul(
                            out=pzrow[0:1, i * 128 : i * 128 + pc],
                            lhsT=z_col[0:pc, c : c + 1],
                            rhs=ident[0:pc, 0:pc],
                            start=True,
                            stop=True,
                        )
                    nc.scalar.copy(
                        z_row[0:1, soff : soff + slen], pzrow[0:1, 0:slen]
                    )
                    nc.tensor.matmul(
                        out=przep[:, 0:slen],
                        lhsT=ones_c[0:1, 0:128],
                        rhs=z_row[0:1, soff : soff + slen],
                        start=True,
                        stop=True,
                    )
                    nc.vector.tensor_copy(
                        zrep[:, soff : soff + slen], przep[:, 0:slen]
                    )

                # PE down part: accumulate over f-chunks; copy out tile-wise
                # as the last chunk's matmuls retire.
                for c in range(NCF):
                    pc = 128 if c < NCF - 1 else LASTF
                    last = c == NCF - 1
                    for ti, (toff, tlen) in enumerate(DT):
                        nc.tensor.matmul(
                            out=pdp[0:1, toff : toff + tlen],
                            lhsT=z_col[0:pc, c : c + 1],
                            rhs=wdp_sb[0:pc, c * DP + toff : c * DP + toff + tlen],
                            start=(c == 0),
                            stop=last,
                        )
                        if last:
                            sl = slice(toff, toff + tlen)
                            if ti % 2 == 0:
                                nc.scalar.copy(osb[0:1, sl], pdp[0:1, sl])
                            else:
                                nc.vector.tensor_copy(osb[0:1, sl], pdp[0:1, sl])

                # DVE down part
                for g in range(NDVG):
                    nc.vector.affine_mul_reduce(
                        out=dve_scr[:, 0:FSH],
                        accum_out=outd_sb[:, g : g + 1],
                        in0=wdd_sb[:, g * FSH : (g + 1) * FSH],
                        in1=zrep[:],
                        scale=1.0,
                        bias=0.0,
                    )

            nc.sync.dma_start(out=outp_d.ap(), in_=osb[:])
            nc.sync.dma_start(out=outd_d.ap(), in_=outd_sb[:])

    nc.compile()
    return nc


def _get_nc():
    if "nc" not in _CACHE:
        _CACHE["nc"] = _build_nc()
    return _CACHE["nc"]


def _q8(W):
    return np.clip(
        np.asarray(W, dtype=np.float32) * WSCL, -15.5, 15.5
    ).astype(NF8)


def make_in_maps(x, Wup, Wgatet, Wdownt, threshold):
    """Shard full inputs into the 8 per-core input maps."""
    x_flat = np.asarray(x, dtype=np.float32).reshape(D)
    xc = np.ascontiguousarray(x_flat.reshape(NCD, 128).T).astype(NF16)
    thr = np.asarray(threshold, dtype=np.float32).reshape(1)
    Wup = np.asarray(Wup, dtype=np.float32)
    Wgatet = np.asarray(Wgatet, dtype=np.float32)
    Wdownt = np.asarray(Wdownt, dtype=np.float32)
    in_maps = []
    for i in range(NCORES):
        sl = slice(i * FSH, (i + 1) * FSH)
        wg_slice = Wgatet[:, sl]                  # [D, FSH] d-major
        wu_slice = Wup[sl, :]                     # [FSH, D] f-major
        wd_slice = Wdownt[sl, :]                  # [FSH, D] f-major

        wg = (
            wg_slice.reshape(NCD, 128, FSH)
            .transpose(1, 0, 2)
            .reshape(128, NCD * FSH)
            .astype(NF16)
        )
        wuT = np.ascontiguousarray(wu_slice.T)    # [D, FSH] d-major
        wu = _q8(
            wuT.reshape(NCD, 128, FSH).transpose(1, 0, 2).reshape(128, NCD * FSH)
        )
        wd_pad = np.zeros((FPAD, DP), dtype=np.float32)
        wd_pad[:FSH] = wd_slice[:, :DP]
        wdp = _q8(
            wd_pad.reshape(NCF, 128, DP).transpose(1, 0, 2).reshape(128, NCF * DP)
        )
        wdT = np.ascontiguousarray(wd_slice[:, DP:].T)  # [D-DP, FSH] d-major
        wdd = _q8(
            wdT.reshape(NDVG, 128, FSH).transpose(1, 0, 2).reshape(128, NDVG * FSH)
        )
        in_maps.append(
            {
                "xc": xc,
                "wu": np.ascontiguousarray(wu),
                "wg": np.ascontiguousarray(wg),
                "wdp": np.ascontiguousarray(wdp),
                "wdd": np.ascontiguousarray(wdd),
                "thr": thr,
            }
        )
    return in_maps


def run_sharded(x, Wup, Wgatet, Wdownt, threshold, trace=False, tmpdir=None):
    """Run on the 8 NeuronCores; returns (full_output, BassKernelResults)."""
    nc = _get_nc()
    in_maps = make_in_maps(x, Wup, Wgatet, Wdownt, threshold)
    res = run_bass_kernel_spmd(
        nc, in_maps, list(range(NCORES)), trace=trace, tmpdir=tmpdir
    )
    acc = np.zeros(D, dtype=np.float64)
    for r in res.results:
        acc[:DP] += r["outp"].reshape(DP).astype(np.float64)
        acc[DP:] += r["outd"].T.reshape(D - DP).astype(np.float64)
    out = acc.astype(np.float32).reshape(1, 1, D)
    return out, res


def kernel(x, Wup, Wgatet, Wdownt, threshold):
    out, _ = run_sharded(x, Wup, Wgatet, Wdownt, threshold)
    return out
